# revision 1
# baseline (speedup 1.0000x reference)
"""MinGRU layer kernel for 8 Trainium2 NeuronCores.

Problem: x (4, 8192, 1024) f32; Wz, Wh (1024, 1024); bz, bh (1024,)
    z = sigmoid(x @ Wz + bz); h_tilde = x @ Wh + bh
    h_t = (1 - z_t) * h_{t-1} + z_t * h_tilde_t   (scan over seq, h_{-1} = 0)

Sharding: 8 cores = 4 batches x 2 output-dim halves. The scan is
independent per (batch, dim), so each core owns a full-sequence scan for
one batch and 512 of the 1024 output dims -- no cross-core traffic.

Layout: host pre-transposes x to (d_in, seq) fp16 per batch. On device the
matmul keeps W stationary (lhsT = W tile, natural layout) and streams x^T,
producing (d_out, seq) tiles in PSUM -- exactly the layout
tensor_tensor_scan needs (scan runs along the free/seq axis, one recurrence
per partition/dim). ScalarE computes a = sigmoid(-(z_pre)) and
z = sigmoid(z_pre) straight out of PSUM; VectorE fuses b = (h_pre + bh) * z
and then runs the scan. Output h^T (512, 8192) f32 is written contiguously;
the host transposes back during the gather.
"""

import sys

if "/opt/trn_rl_repo" not in sys.path:
    sys.path.insert(0, "/opt/trn_rl_repo")

import numpy as np

from concourse import bass, mybir
from concourse.tile import TileContext
from concourse.bass_utils import run_bass_kernel_spmd

BATCH, SEQ, D = 4, 8192, 1024
DH = 512            # output dims per core
N_CORES = 8
# Seq chunk schedule: small chunks first so the PE starts on real work
# early (warms the HAM clock gate) and the consumer engines ramp before
# the PE hits full streaming rate.
CHUNKS = [256, 256, 512] + [1024] * 6 + [512, 256, 128, 128]
assert sum(CHUNKS) == SEQ
NCHUNK = len(CHUNKS)
CHUNK_MAX = max(CHUNKS)
NM = DH // 128      # output-dim tiles per core
NK = D // 128       # contraction tiles

F16 = mybir.dt.float16
F32 = mybir.dt.float32
AF = mybir.ActivationFunctionType
OP = mybir.AluOpType


_WAIT_LIMIT = 1  # this walrus build rejects multiple sem waits per instruction


def _split_sync_waits(nc):
    """Move excess semaphore waits (beyond _WAIT_LIMIT) off each instruction
    onto same-engine nops inserted immediately before it. Waits only gate
    execution, so hoisting some onto a preceding nop in the same engine
    stream is semantics-preserving."""
    import bass_rust

    n_extra = 0
    for fn in nc.m.functions:
        for blk in fn.blocks:
            insts = blk.instructions
            out = []
            for inst in insts:
                si = inst.sync_info
                if si is not None and si.on_wait and len(si.on_wait) > _WAIT_LIMIT:
                    waits = list(si.on_wait)
                    head, tail = waits[:-_WAIT_LIMIT], waits[-_WAIT_LIMIT:]
                    for j in range(0, len(head), _WAIT_LIMIT):
                        n_extra += 1
                        nop = bass_rust.InstNoOp(
                            name=f"{inst.name}-waitsplit{j}",
                            engine=inst.engine,
                            sync_info=type(si)(
                                on_wait=head[j:j + _WAIT_LIMIT], on_update=[]
                            ),
                            bass_nofuse=True,
                        )
                        nc.register_instruction(nop, overwrite=True)
                        out.append(nop)
                    si.on_wait = tail
                out.append(inst)
            if n_extra:
                blk.instructions = out
    return n_extra


def _build_program():
    nc = bass.Bass("TRN2", target_bir_lowering=False, debug=False)

    xT = nc.dram_tensor("xT", [D, SEQ], F16, kind="ExternalInput").ap()
    wz = nc.dram_tensor("wz", [D, DH], F16, kind="ExternalInput").ap()
    wh = nc.dram_tensor("wh", [D, DH], F16, kind="ExternalInput").ap()
    # biases packed: [bz | bzn | bh] x NM m-tiles -> (128, 3*NM), one DMA
    bias = nc.dram_tensor("bias", [128, 3 * NM], F32, kind="ExternalInput").ap()
    hT = nc.dram_tensor("hT", [DH, SEQ], F32, kind="ExternalOutput").ap()

    with TileContext(nc) as tc:
        with (
            tc.tile_pool(name="weights", bufs=1) as wpool,
            tc.tile_pool(name="bias", bufs=1) as biaspool,
            tc.tile_pool(name="xt", bufs=4) as xpool,
            tc.tile_pool(name="a", bufs=4) as apool,
            tc.tile_pool(name="z", bufs=4) as zpool,
            tc.tile_pool(name="b", bufs=4) as bpool,
            tc.tile_pool(name="h", bufs=4) as hpool,
            tc.tile_pool(name="psz", bufs=4, space="PSUM") as pszpool,
            tc.tile_pool(name="psh", bufs=4, space="PSUM") as pshpool,
        ):
            # Weights resident for the whole kernel: (128 k, 512 m) per
            # k-tile. Weight/bias/output DMAs ride the SWDGE (gpsimd) path;
            # the sync HWDGE ring is dedicated to x^T prefetch and the
            # scalar ring stays free for ACT compute.
            # Bias first on the scalar HWDGE ring: ACT's first sigmoid needs
            # it, and on SWDGE it queues behind 15 weight-DMA issues and
            # lands ~10us late, stalling every consumer at ramp time.
            bias_t = biaspool.tile([128, 3 * NM], F32, tag="bias")
            nc.scalar.dma_start(out=bias_t[:], in_=bias[:])
            bz_t = [bias_t[:, m:m + 1] for m in range(NM)]
            bzn_t = [bias_t[:, NM + m:NM + m + 1] for m in range(NM)]
            bh_t = [bias_t[:, 2 * NM + m:2 * NM + m + 1] for m in range(NM)]

            wz_b, wh_b = [], []
            for kt in range(NK):
                w1 = wpool.tile([128, DH], F16, tag=f"wz{kt}")
                # k-tile 0 gates the first LDWEIGHTS: give it the
                # low-latency HWDGE sync ring ahead of the x^T stream.
                eng = nc.sync if kt == 0 else nc.gpsimd
                eng.dma_start(out=w1[:], in_=wz[kt * 128:(kt + 1) * 128, :])
                wz_b.append(w1)
            for kt in range(NK):
                w2 = wpool.tile([128, DH], F16, tag=f"wh{kt}")
                nc.gpsimd.dma_start(out=w2[:], in_=wh[kt * 128:(kt + 1) * 128, :])
                wh_b.append(w2)
            wz_t = [[wz_b[kt][:, m * 128:(m + 1) * 128] for m in range(NM)]
                    for kt in range(NK)]
            wh_t = [[wh_b[kt][:, m * 128:(m + 1) * 128] for m in range(NM)]
                    for kt in range(NK)]

            last_h = [None] * NM
            seq_off = 0
            for c in range(NCHUNK):
                chunk = CHUNKS[c]
                xt = []
                for kt in range(NK):
                    t = xpool.tile([128, CHUNK_MAX], F16, tag=f"x{kt}")
                    nc.sync.dma_start(
                        out=t[:, :chunk],
                        in_=xT[kt * 128:(kt + 1) * 128,
                               seq_off:seq_off + chunk],
                    )
                    xt.append(t)

                h_big = []
                for m in range(NM):
                    h_m = hpool.tile([128, CHUNK_MAX], F32, tag=f"h{m}")
                    h_big.append(h_m)
                bounds = []
                acc = 0
                while acc < chunk:
                    bounds.append((acc, min(chunk, acc + 512)))
                    acc = min(chunk, acc + 512)
                for w0, w1 in bounds:
                    for m in range(NM):
                        psz = pszpool.tile([128, 512], F32)
                        psh = pshpool.tile([128, 512], F32)
                        for kt in range(NK):
                            nc.tensor.matmul(
                                psz[:, :w1 - w0],
                                wz_t[kt][m][:],
                                xt[kt][:, w0:w1],
                                start=(kt == 0),
                                stop=(kt == NK - 1),
                            )
                        for kt in range(NK):
                            nc.tensor.matmul(
                                psh[:, :w1 - w0],
                                wh_t[kt][m][:],
                                xt[kt][:, w0:w1],
                                start=(kt == 0),
                                stop=(kt == NK - 1),
                            )
                        # z first: the DVE multiply consumes it, so z-then-a
                        # shortens the STT->scan critical path by one ACT op.
                        z_t = zpool.tile([128, 512], F32)
                        nc.scalar.activation(z_t[:, :w1 - w0], psz[:, :w1 - w0],
                                             AF.Sigmoid,
                                             bias=bz_t[m][:], scale=1.0)
                        # a = 1 - sigmoid(z_pre + bz) = sigmoid(-z_pre - bz)
                        a_t = apool.tile([128, 512], F32)
                        nc.scalar.activation(a_t[:, :w1 - w0], psz[:, :w1 - w0],
                                             AF.Sigmoid,
                                             bias=bzn_t[m][:], scale=-1.0)
                        # b = (h_pre + bh) * z
                        b_t = bpool.tile([128, 512], F32)
                        nc.vector.scalar_tensor_tensor(
                            b_t[:, :w1 - w0], psh[:, :w1 - w0], bh_t[m][:],
                            z_t[:, :w1 - w0],
                            op0=OP.add, op1=OP.mult,
                        )
                        # h_t = a_t * h_{t-1} + b_t along seq
                        h_t = h_big[m][:, w0:w1]
                        init = 0.0 if last_h[m] is None else last_h[m][:, -1:]
                        nc.vector.tensor_tensor_scan(
                            h_t, a_t[:, :w1 - w0], b_t[:, :w1 - w0], init,
                            op0=OP.mult, op1=OP.add,
                        )
                        last_h[m] = h_t
                # Final chunk's outputs go via HWDGE (sync) -- the SWDGE
                # path adds a slow GpSimd drain right at the kernel tail --
                # and at s5 granularity so earlier pieces flush during the
                # last scans.
                if c == NCHUNK - 1:
                    # Spread final flushes over both HWDGE issuers so the
                    # issue latency doesn't serialize at the tail.
                    tail_eng = [nc.sync, nc.scalar, nc.sync, nc.scalar]
                    for mm in range(NM):
                        for w0, w1 in bounds:
                            tail_eng[mm].dma_start(
                                out=hT[mm * 128:(mm + 1) * 128,
                                       seq_off + w0:seq_off + w1],
                                in_=h_big[mm][:, w0:w1],
                            )
                else:
                    for mm in range(NM):
                        nc.gpsimd.dma_start(
                            out=hT[mm * 128:(mm + 1) * 128,
                                   seq_off:seq_off + chunk],
                            in_=h_big[mm][:, :chunk],
                        )
                seq_off += chunk
    _split_sync_waits(nc)
    return nc


_NC_CACHE = None


def _get_program():
    global _NC_CACHE
    if _NC_CACHE is None:
        _NC_CACHE = _build_program()
    return _NC_CACHE


def _make_in_maps(x, Wz, bz, Wh, bh):
    xT16 = [np.ascontiguousarray(x[b].T).astype(np.float16) for b in range(BATCH)]
    wzh = [np.ascontiguousarray(Wz[:, c * DH:(c + 1) * DH]).astype(np.float16)
           for c in range(2)]
    whh = [np.ascontiguousarray(Wh[:, c * DH:(c + 1) * DH]).astype(np.float16)
           for c in range(2)]
    # bias[p, m] = bz[m*128+p]; columns [0:NM]=bz, [NM:2NM]=-bz, [2NM:3NM]=bh
    biases = []
    for c in range(2):
        bzc = bz[c * DH:(c + 1) * DH].astype(np.float32).reshape(NM, 128).T
        bhc = bh[c * DH:(c + 1) * DH].astype(np.float32).reshape(NM, 128).T
        biases.append(np.ascontiguousarray(np.hstack([bzc, -bzc, bhc])))
    in_maps = []
    for i in range(N_CORES):
        b, c = i // 2, i % 2
        in_maps.append({
            "xT": xT16[b], "wz": wzh[c], "wh": whh[c], "bias": biases[c],
        })
    return in_maps


def _run(x, Wz, bz, Wh, bh, trace=False, trace_cores=None):
    import time

    nc = _get_program()
    in_maps = _make_in_maps(x, Wz, bz, Wh, bh)
    res = None
    for attempt in range(3):
        try:
            res = run_bass_kernel_spmd(
                nc, in_maps, list(range(N_CORES)),
                trace=trace, trace_cores=trace_cores,
            )
            break
        except Exception:
            # Transient NRT device errors have been observed on the first
            # execution after a fresh compile; retry.
            if attempt == 2:
                raise
            time.sleep(10)
    out = np.empty((BATCH, SEQ, D), dtype=np.float32)
    for i in range(N_CORES):
        b, c = i // 2, i % 2
        out[b, :, c * DH:(c + 1) * DH] = res.results[i]["hT"].T
    return out, res


def kernel(x, Wz, bz, Wh, bh):
    x = np.asarray(x, dtype=np.float32)
    Wz = np.asarray(Wz, dtype=np.float32)
    Wh = np.asarray(Wh, dtype=np.float32)
    bz = np.asarray(bz, dtype=np.float32)
    bh = np.asarray(bh, dtype=np.float32)
    out, _ = _run(x, Wz, bz, Wh, bh, trace=False)
    return out



# revision 2
# speedup vs baseline: 1.2378x; 1.2378x over previous
"""MinGRU layer kernel for 8 Trainium2 NeuronCores.

Problem: x (4, 8192, 1024) f32; Wz, Wh (1024, 1024); bz, bh (1024,)
    z = sigmoid(x @ Wz + bz); h_tilde = x @ Wh + bh
    h_t = (1 - z_t) * h_{t-1} + z_t * h_tilde_t   (scan over seq, h_{-1} = 0)

Sharding: 8 cores = 4 batches x 2 output-dim halves. The scan is
independent per (batch, dim), so each core owns a full-sequence scan for
one batch and 512 of the 1024 output dims -- no cross-core traffic.

Precision: the z-path matmul runs in fp8 e4m3 DoubleRow mode (2x PE
throughput; K=256 per instruction). Wz is pre-scaled by 32 on the host so
its values sit in e4m3's normal range; the ACT sigmoid compensates with
scale=1/32. The h-path matmul stays fp16 -- fp8 there would push the L2
error (4.1e-2) over the 2e-2 gate, while z-only-fp8 measures 1.74e-2.
PE time: 8192*1024*512*2 MACs fp16 (109.3us) + same in fp8 at 2x
(54.6us) = 164us floor vs 218.6us all-fp16.

Layout: host pre-transposes x to (d_in, seq) in fp16 AND fp8 per batch.
On device the matmul keeps W stationary and streams x^T, producing
(d_out, seq) tiles in PSUM -- the layout tensor_tensor_scan needs.
ScalarE computes z = sigmoid(psz/32 + bz) and a = sigmoid(-psz/32 - bz)
straight out of PSUM; VectorE fuses b = (h_pre + bh) * z and runs the
scan. Output h^T (512, 8192) f32 is written contiguously; the host
transposes back during the gather.
"""

import sys

if "/opt/trn_rl_repo" not in sys.path:
    sys.path.insert(0, "/opt/trn_rl_repo")

import numpy as np

from concourse import bass, mybir
from concourse.tile import TileContext
from concourse.bass_utils import run_bass_kernel_spmd

BATCH, SEQ, D = 4, 8192, 1024
DH = 512            # output dims per core
N_CORES = 8
# Seq chunk schedule: small chunks first so the PE starts on real work
# early (warms the HAM clock gate) and the consumer engines ramp before
# the PE hits full streaming rate.
CHUNKS = [256, 256, 512] + [1024] * 6 + [512, 256, 128, 128]
assert sum(CHUNKS) == SEQ
CHUNK_MAX = max(CHUNKS)
NM = DH // 128      # output-dim tiles per core
NK = D // 128       # contraction tiles (fp16 h-path)
NK2 = D // 256      # DoubleRow contraction tiles (fp8 z-path)

F8 = mybir.dt.float8e4
F16 = mybir.dt.float16
F32 = mybir.dt.float32
AF = mybir.ActivationFunctionType
OP = mybir.AluOpType
DR = mybir.MatmulPerfMode.DoubleRow

WZ_SCALE = 32.0     # host multiplies Wz by this before the e4m3 cast


_WAIT_LIMIT = 1  # this walrus build rejects multiple sem waits per instruction


def _split_sync_waits(nc):
    """Move excess semaphore waits (beyond _WAIT_LIMIT) off each instruction
    onto same-engine nops inserted immediately before it. Waits only gate
    execution, so hoisting some onto a preceding nop in the same engine
    stream is semantics-preserving."""
    import bass_rust

    n_extra = 0
    for fn in nc.m.functions:
        for blk in fn.blocks:
            insts = blk.instructions
            out = []
            for inst in insts:
                si = inst.sync_info
                if si is not None and si.on_wait and len(si.on_wait) > _WAIT_LIMIT:
                    waits = list(si.on_wait)
                    head, tail = waits[:-_WAIT_LIMIT], waits[-_WAIT_LIMIT:]
                    for j in range(0, len(head), _WAIT_LIMIT):
                        n_extra += 1
                        nop = bass_rust.InstNoOp(
                            name=f"{inst.name}-waitsplit{j}",
                            engine=inst.engine,
                            sync_info=type(si)(
                                on_wait=head[j:j + _WAIT_LIMIT], on_update=[]
                            ),
                            bass_nofuse=True,
                        )
                        nc.register_instruction(nop, overwrite=True)
                        out.append(nop)
                    si.on_wait = tail
                out.append(inst)
            if n_extra:
                blk.instructions = out
    return n_extra


def _build_program(chunks=CHUNKS):
    seq = sum(chunks)
    nchunk = len(chunks)
    chunk_max = max(chunks)

    nc = bass.Bass("TRN2", target_bir_lowering=False, debug=False)

    xT = nc.dram_tensor("xT", [D, seq], F16, kind="ExternalInput").ap()
    xT8 = nc.dram_tensor("xT8", [D, seq], F8, kind="ExternalInput").ap()
    wz8 = nc.dram_tensor("wz8", [D, DH], F8, kind="ExternalInput").ap()
    wh = nc.dram_tensor("wh", [D, DH], F16, kind="ExternalInput").ap()
    # biases packed: [bz | bzn | bh] x NM m-tiles -> (128, 3*NM), one DMA
    bias = nc.dram_tensor("bias", [128, 3 * NM], F32, kind="ExternalInput").ap()
    hT = nc.dram_tensor("hT", [DH, seq], F32, kind="ExternalOutput").ap()

    # (d, s) indexed as d = i*128 + p  ->  (p, i, s): partition p holds
    # k-rows {p, 128+p, ...}; free dim i selects the 128-row k-subtile.
    xT_r = xT.rearrange("(i p) t -> p i t", p=128)
    xT8_r = xT8.rearrange("(i p) t -> p i t", p=128)
    wz8_r = wz8.rearrange("(i p) m -> p i m", p=128)

    with TileContext(nc) as tc:
        with (
            tc.tile_pool(name="weights", bufs=1) as wpool,
            tc.tile_pool(name="bias", bufs=1) as biaspool,
            tc.tile_pool(name="xt", bufs=4) as xpool,
            tc.tile_pool(name="x8t", bufs=4) as x8pool,
            tc.tile_pool(name="a", bufs=4) as apool,
            tc.tile_pool(name="z", bufs=4) as zpool,
            tc.tile_pool(name="b", bufs=4) as bpool,
            tc.tile_pool(name="h", bufs=4) as hpool,
            tc.tile_pool(name="psz", bufs=4, space="PSUM") as pszpool,
            tc.tile_pool(name="psh", bufs=4, space="PSUM") as pshpool,
        ):
            # Bias first on the scalar HWDGE ring: ACT's first sigmoid needs
            # it, and on SWDGE it queues behind the weight DMAs.
            bias_t = biaspool.tile([128, 3 * NM], F32, tag="bias")
            nc.scalar.dma_start(out=bias_t[:], in_=bias[:])
            bz_t = [bias_t[:, m:m + 1] for m in range(NM)]
            bzn_t = [bias_t[:, NM + m:NM + m + 1] for m in range(NM)]
            bh_t = [bias_t[:, 2 * NM + m:2 * NM + m + 1] for m in range(NM)]

            # fp8 z-weights: one [128, 2, DH] tile per DoubleRow k-tile
            # (k-groups p and 128+p of the 256-row block side by side).
            # Small (0.5 MB total) and they gate the first matmul: load on
            # the low-latency sync HWDGE ring ahead of the x stream.
            wz8_b = []
            for kt in range(NK2):
                w1 = wpool.tile([128, 2, DH], F8, tag=f"wz8_{kt}")
                nc.sync.dma_start(out=w1[:], in_=wz8_r[:, 2 * kt:2 * kt + 2, :])
                wz8_b.append(w1)
            # fp16 h-weights: per 128-row k-tile, SWDGE (gpsimd) path keeps
            # the sync ring free for x prefetch.
            wh_b = []
            for kt in range(NK):
                w2 = wpool.tile([128, DH], F16, tag=f"wh{kt}")
                nc.gpsimd.dma_start(out=w2[:], in_=wh[kt * 128:(kt + 1) * 128, :])
                wh_b.append(w2)
            wz8_t = [[wz8_b[kt][:, :, m * 128:(m + 1) * 128] for m in range(NM)]
                     for kt in range(NK2)]
            wh_t = [[wh_b[kt][:, m * 128:(m + 1) * 128] for m in range(NM)]
                    for kt in range(NK)]

            last_h = [None] * NM
            seq_off = 0
            for c in range(nchunk):
                chunk = chunks[c]
                # One DMA per chunk per precision: [128, 8, chunk] gathers
                # all 8 k-subtiles (innermost run = chunk elems, contiguous).
                xt = xpool.tile([128, NK, chunk_max], F16, tag="x16")
                nc.sync.dma_start(
                    out=xt[:, :, :chunk],
                    in_=xT_r[:, :, seq_off:seq_off + chunk],
                )
                x8t = x8pool.tile([128, NK, chunk_max], F8, tag="x8")
                nc.sync.dma_start(
                    out=x8t[:, :, :chunk],
                    in_=xT8_r[:, :, seq_off:seq_off + chunk],
                )

                h_big = []
                for m in range(NM):
                    h_m = hpool.tile([128, chunk_max], F32, tag=f"h{m}")
                    h_big.append(h_m)
                bounds = []
                acc = 0
                while acc < chunk:
                    bounds.append((acc, min(chunk, acc + 512)))
                    acc = min(chunk, acc + 512)
                for w0, w1 in bounds:
                    for m in range(NM):
                        psz = pszpool.tile([128, 512], F32)
                        psh = pshpool.tile([128, 512], F32)
                        # z-path: fp8 DoubleRow, K=256 per matmul, 2x rate
                        for kt in range(NK2):
                            nc.tensor.matmul(
                                psz[:, :w1 - w0],
                                wz8_t[kt][m],
                                x8t[:, 2 * kt:2 * kt + 2, w0:w1],
                                start=(kt == 0),
                                stop=(kt == NK2 - 1),
                                perf_mode=DR,
                            )
                        # h-path: fp16
                        for kt in range(NK):
                            nc.tensor.matmul(
                                psh[:, :w1 - w0],
                                wh_t[kt][m],
                                xt[:, kt, w0:w1],
                                start=(kt == 0),
                                stop=(kt == NK - 1),
                            )
                        # z first: the DVE multiply consumes it, so z-then-a
                        # shortens the STT->scan critical path by one ACT op.
                        # psz holds 32*z_pre (Wz host-scaled); ACT scale
                        # compensates.
                        z_t = zpool.tile([128, 512], F32)
                        nc.scalar.activation(z_t[:, :w1 - w0], psz[:, :w1 - w0],
                                             AF.Sigmoid,
                                             bias=bz_t[m][:], scale=1.0 / WZ_SCALE)
                        # a = 1 - sigmoid(z_pre + bz) = sigmoid(-z_pre - bz)
                        a_t = apool.tile([128, 512], F32)
                        nc.scalar.activation(a_t[:, :w1 - w0], psz[:, :w1 - w0],
                                             AF.Sigmoid,
                                             bias=bzn_t[m][:], scale=-1.0 / WZ_SCALE)
                        # b = (h_pre + bh) * z
                        b_t = bpool.tile([128, 512], F32)
                        nc.vector.scalar_tensor_tensor(
                            b_t[:, :w1 - w0], psh[:, :w1 - w0], bh_t[m][:],
                            z_t[:, :w1 - w0],
                            op0=OP.add, op1=OP.mult,
                        )
                        # h_t = a_t * h_{t-1} + b_t along seq
                        h_t = h_big[m][:, w0:w1]
                        init = 0.0 if last_h[m] is None else last_h[m][:, -1:]
                        nc.vector.tensor_tensor_scan(
                            h_t, a_t[:, :w1 - w0], b_t[:, :w1 - w0], init,
                            op0=OP.mult, op1=OP.add,
                        )
                        last_h[m] = h_t
                # Final chunk's outputs go via HWDGE (sync) -- the SWDGE
                # path adds a slow GpSimd drain right at the kernel tail --
                # and at s5 granularity so earlier pieces flush during the
                # last scans.
                if c == nchunk - 1:
                    # Spread final flushes over both HWDGE issuers so the
                    # issue latency doesn't serialize at the tail.
                    tail_eng = [nc.sync, nc.scalar, nc.sync, nc.scalar]
                    for mm in range(NM):
                        for w0, w1 in bounds:
                            tail_eng[mm].dma_start(
                                out=hT[mm * 128:(mm + 1) * 128,
                                       seq_off + w0:seq_off + w1],
                                in_=h_big[mm][:, w0:w1],
                            )
                else:
                    for mm in range(NM):
                        nc.gpsimd.dma_start(
                            out=hT[mm * 128:(mm + 1) * 128,
                                   seq_off:seq_off + chunk],
                            in_=h_big[mm][:, :chunk],
                        )
                seq_off += chunk
    _split_sync_waits(nc)
    return nc


_NC_CACHE = None


def _get_program():
    global _NC_CACHE
    if _NC_CACHE is None:
        _NC_CACHE = _build_program()
    return _NC_CACHE


def _make_in_maps(x, Wz, bz, Wh, bh):
    import ml_dtypes

    f8np = ml_dtypes.float8_e4m3
    xT16 = [np.ascontiguousarray(x[b].T).astype(np.float16) for b in range(BATCH)]
    xT8 = [np.ascontiguousarray(x[b].T).astype(f8np) for b in range(BATCH)]
    wz8h = [np.ascontiguousarray(
                Wz[:, c * DH:(c + 1) * DH] * WZ_SCALE).astype(f8np)
            for c in range(2)]
    whh = [np.ascontiguousarray(Wh[:, c * DH:(c + 1) * DH]).astype(np.float16)
           for c in range(2)]
    # bias[p, m] = bz[m*128+p]; columns [0:NM]=bz, [NM:2NM]=-bz, [2NM:3NM]=bh
    biases = []
    for c in range(2):
        bzc = bz[c * DH:(c + 1) * DH].astype(np.float32).reshape(NM, 128).T
        bhc = bh[c * DH:(c + 1) * DH].astype(np.float32).reshape(NM, 128).T
        biases.append(np.ascontiguousarray(np.hstack([bzc, -bzc, bhc])))
    in_maps = []
    for i in range(N_CORES):
        b, c = i // 2, i % 2
        in_maps.append({
            "xT": xT16[b], "xT8": xT8[b], "wz8": wz8h[c], "wh": whh[c],
            "bias": biases[c],
        })
    return in_maps


def _run(x, Wz, bz, Wh, bh, trace=False, trace_cores=None):
    import time

    nc = _get_program()
    in_maps = _make_in_maps(x, Wz, bz, Wh, bh)
    res = None
    for attempt in range(3):
        try:
            res = run_bass_kernel_spmd(
                nc, in_maps, list(range(N_CORES)),
                trace=trace, trace_cores=trace_cores,
            )
            break
        except Exception:
            # Transient NRT device errors have been observed on the first
            # execution after a fresh compile; retry.
            if attempt == 2:
                raise
            time.sleep(10)
    out = np.empty((BATCH, SEQ, D), dtype=np.float32)
    for i in range(N_CORES):
        b, c = i // 2, i % 2
        out[b, :, c * DH:(c + 1) * DH] = res.results[i]["hT"].T
    return out, res


def kernel(x, Wz, bz, Wh, bh):
    x = np.asarray(x, dtype=np.float32)
    Wz = np.asarray(Wz, dtype=np.float32)
    Wh = np.asarray(Wh, dtype=np.float32)
    bz = np.asarray(bz, dtype=np.float32)
    bh = np.asarray(bh, dtype=np.float32)
    out, _ = _run(x, Wz, bz, Wh, bh, trace=False)
    return out


# revision 3
# speedup vs baseline: 1.2578x; 1.0162x over previous
"""MinGRU layer kernel for 8 Trainium2 NeuronCores.

Problem: x (4, 8192, 1024) f32; Wz, Wh (1024, 1024); bz, bh (1024,)
    z = sigmoid(x @ Wz + bz); h_tilde = x @ Wh + bh
    h_t = (1 - z_t) * h_{t-1} + z_t * h_tilde_t   (scan over seq, h_{-1} = 0)

Sharding: 8 cores = 4 batches x 2 output-dim halves. The scan is
independent per (batch, dim), so each core owns a full-sequence scan for
one batch and 512 of the 1024 output dims -- no cross-core traffic.

Precision: the z-path matmul runs in fp8 e4m3 DoubleRow mode (2x PE
throughput; K=256 per instruction). Wz is pre-scaled by 32 on the host so
its values sit in e4m3's normal range; the ACT sigmoid compensates with
scale=1/32. The h-path matmul stays fp16 -- fp8 there would push the L2
error (4.1e-2) over the 2e-2 gate, while z-only-fp8 measures 1.74e-2.
PE floor: 109.3us (h, fp16) + 54.6us (z, fp8 2x) = 164us vs 218.6us
all-fp16.

DMA: the HWDGE hardware queue costs ~4.3ns per line (contiguous run), so
the host packs x per (chunk, partition) -- each chunk transfer is 128
lines of 8*chunk elems instead of 1024 short lines. Same for the output:
the four m-tiles share one [128, 4, chunk] fp16 tile stored as one
contiguous 128-line DMA per chunk (host unpacks + upcasts). All stores
ride HWDGE; SWDGE (gpsimd) only carries half the Wh load, avoiding its
slow tail drain.
"""

import sys

if "/opt/trn_rl_repo" not in sys.path:
    sys.path.insert(0, "/opt/trn_rl_repo")

import numpy as np

from concourse import bass, mybir
from concourse.tile import TileContext
from concourse.bass_utils import run_bass_kernel_spmd

BATCH, SEQ, D = 4, 8192, 1024
DH = 512            # output dims per core
N_CORES = 8
# Seq chunk schedule: small chunks first so the PE starts on real work
# early (warms the HAM clock gate) and the consumer engines ramp before
# the PE hits full streaming rate.
CHUNKS = [256, 256, 512] + [1024] * 6 + [512, 256, 128, 128]
assert sum(CHUNKS) == SEQ
CHUNK_MAX = max(CHUNKS)
NM = DH // 128      # output-dim tiles per core
NK = D // 128      # contraction tiles (fp16 h-path)
NK2 = D // 256      # DoubleRow contraction tiles (fp8 z-path)

F8 = mybir.dt.float8e4
F16 = mybir.dt.float16
F32 = mybir.dt.float32
AF = mybir.ActivationFunctionType
OP = mybir.AluOpType
DR = mybir.MatmulPerfMode.DoubleRow

WZ_SCALE = 32.0     # host multiplies Wz by this before the e4m3 cast


_WAIT_LIMIT = 1  # this walrus build rejects multiple sem waits per instruction


def _split_sync_waits(nc):
    """Move excess semaphore waits (beyond _WAIT_LIMIT) off each instruction
    onto same-engine nops inserted immediately before it. Waits only gate
    execution, so hoisting some onto a preceding nop in the same engine
    stream is semantics-preserving."""
    import bass_rust

    n_extra = 0
    for fn in nc.m.functions:
        for blk in fn.blocks:
            insts = blk.instructions
            out = []
            for inst in insts:
                si = inst.sync_info
                if si is not None and si.on_wait and len(si.on_wait) > _WAIT_LIMIT:
                    waits = list(si.on_wait)
                    head, tail = waits[:-_WAIT_LIMIT], waits[-_WAIT_LIMIT:]
                    for j in range(0, len(head), _WAIT_LIMIT):
                        n_extra += 1
                        nop = bass_rust.InstNoOp(
                            name=f"{inst.name}-waitsplit{j}",
                            engine=inst.engine,
                            sync_info=type(si)(
                                on_wait=head[j:j + _WAIT_LIMIT], on_update=[]
                            ),
                            bass_nofuse=True,
                        )
                        nc.register_instruction(nop, overwrite=True)
                        out.append(nop)
                    si.on_wait = tail
                out.append(inst)
            if n_extra:
                blk.instructions = out
    return n_extra


def _build_program(chunks=CHUNKS):
    seq = sum(chunks)
    nchunk = len(chunks)
    chunk_max = max(chunks)

    nc = bass.Bass("TRN2", target_bir_lowering=False, debug=False)

    # Host-packed layouts (see _make_in_maps):
    #   xP16/xP8[p, 8*t_off + i*chunk + t] = x[seq_off+t, i*128+p] per chunk
    #   wz8p[p, (2*kt+i)*DH + m]         = Wz[256*kt + 128*i + p, m] * 32
    #   hTp[p, 4*seq_off + m*chunk + t]  = h[seq_off+t, m*128+p]
    xP16 = nc.dram_tensor("xP16", [128, NK * seq], F16, kind="ExternalInput").ap()
    xP8 = nc.dram_tensor("xP8", [128, NK * seq], F8, kind="ExternalInput").ap()
    wz8p = nc.dram_tensor("wz8p", [128, NK * DH // 128 * 128], F8,
                          kind="ExternalInput").ap()
    wh = nc.dram_tensor("wh", [D, DH], F16, kind="ExternalInput").ap()
    # biases packed: [bz | bzn | bh] x NM m-tiles -> (128, 3*NM), one DMA
    bias = nc.dram_tensor("bias", [128, 3 * NM], F32, kind="ExternalInput").ap()
    hTp = nc.dram_tensor("hTp", [128, NM * seq], F16, kind="ExternalOutput").ap()

    with TileContext(nc) as tc:
        with (
            tc.tile_pool(name="weights", bufs=1) as wpool,
            tc.tile_pool(name="bias", bufs=1) as biaspool,
            tc.tile_pool(name="xt", bufs=4) as xpool,
            tc.tile_pool(name="x8t", bufs=4) as x8pool,
            tc.tile_pool(name="a", bufs=4) as apool,
            tc.tile_pool(name="z", bufs=4) as zpool,
            tc.tile_pool(name="b", bufs=4) as bpool,
            tc.tile_pool(name="h", bufs=4) as hpool,
            tc.tile_pool(name="psz", bufs=4, space="PSUM") as pszpool,
            tc.tile_pool(name="psh", bufs=4, space="PSUM") as pshpool,
        ):
            # Bias first on the scalar HWDGE ring: ACT's first sigmoid needs
            # it early.
            bias_t = biaspool.tile([128, 3 * NM], F32, tag="bias")
            nc.scalar.dma_start(out=bias_t[:], in_=bias[:])
            bz_t = [bias_t[:, m:m + 1] for m in range(NM)]
            bzn_t = [bias_t[:, NM + m:NM + m + 1] for m in range(NM)]
            bh_t = [bias_t[:, 2 * NM + m:2 * NM + m + 1] for m in range(NM)]

            # Wh split across both DMA paths so the 2MB load doesn't gate
            # the first h-matmuls: k0-3 on the scalar HWDGE ring (fast
            # queue), k4-7 on SWDGE (parallel queue, lands ~12us).
            wh_b = []
            for kt in range(NK):
                w2 = wpool.tile([128, DH], F16, tag=f"wh{kt}")
                eng = nc.scalar if kt < 4 else nc.gpsimd
                eng.dma_start(out=w2[:], in_=wh[kt * 128:(kt + 1) * 128, :])
                wh_b.append(w2)
            # fp8 z-weights: single packed DMA; tile dim1 = (kt, i) pairs,
            # slice [:, 2kt:2kt+2, :] is one DoubleRow k-tile.
            wz8_t = wpool.tile([128, 2 * NK2, DH], F8, tag="wz8")
            nc.scalar.dma_start(out=wz8_t[:], in_=wz8p[:])
            wh_t = [[wh_b[kt][:, m * 128:(m + 1) * 128] for m in range(NM)]
                    for kt in range(NK)]

            last_h = [None] * NM
            seq_off = 0
            for c in range(nchunk):
                chunk = chunks[c]
                # One packed DMA per chunk per precision: 128 contiguous
                # lines of 8*chunk elems each.
                xt = xpool.tile([128, NK * chunk_max], F16, tag="x16")
                nc.sync.dma_start(
                    out=xt[:, :NK * chunk],
                    in_=xP16[:, NK * seq_off:NK * (seq_off + chunk)],
                )
                x8t = x8pool.tile([128, NK * chunk_max], F8, tag="x8")
                nc.sync.dma_start(
                    out=x8t[:, :NK * chunk],
                    in_=xP8[:, NK * seq_off:NK * (seq_off + chunk)],
                )
                xv = xt[:, :NK * chunk].rearrange("p (i t) -> p i t", i=NK)
                xv8 = x8t[:, :NK * chunk].rearrange("p (i t) -> p i t", i=NK)

                # All 4 m-tiles share one fp16 h tile -> one packed
                # contiguous store per chunk.
                h_t3 = hpool.tile([128, NM * chunk_max], F16, tag="h")
                hv = h_t3[:, :NM * chunk].rearrange("p (m t) -> p m t", m=NM)

                bounds = []
                acc = 0
                while acc < chunk:
                    bounds.append((acc, min(chunk, acc + 512)))
                    acc = min(chunk, acc + 512)
                for w0, w1 in bounds:
                    for m in range(NM):
                        psz = pszpool.tile([128, 512], F32)
                        psh = pshpool.tile([128, 512], F32)
                        # z-path: fp8 DoubleRow, K=256 per matmul, 2x rate
                        for kt in range(NK2):
                            nc.tensor.matmul(
                                psz[:, :w1 - w0],
                                wz8_t[:, 2 * kt:2 * kt + 2, m * 128:(m + 1) * 128],
                                xv8[:, 2 * kt:2 * kt + 2, w0:w1],
                                start=(kt == 0),
                                stop=(kt == NK2 - 1),
                                perf_mode=DR,
                            )
                        # h-path: fp16
                        for kt in range(NK):
                            nc.tensor.matmul(
                                psh[:, :w1 - w0],
                                wh_t[kt][m],
                                xv[:, kt, w0:w1],
                                start=(kt == 0),
                                stop=(kt == NK - 1),
                            )
                        # z first: the DVE multiply consumes it, so z-then-a
                        # shortens the STT->scan critical path by one ACT op.
                        # psz holds 32*z_pre (Wz host-scaled); ACT scale
                        # compensates.
                        z_t = zpool.tile([128, 512], F32)
                        nc.scalar.activation(z_t[:, :w1 - w0], psz[:, :w1 - w0],
                                             AF.Sigmoid,
                                             bias=bz_t[m][:], scale=1.0 / WZ_SCALE)
                        # a = 1 - sigmoid(z_pre + bz) = sigmoid(-z_pre - bz)
                        a_t = apool.tile([128, 512], F32)
                        nc.scalar.activation(a_t[:, :w1 - w0], psz[:, :w1 - w0],
                                             AF.Sigmoid,
                                             bias=bzn_t[m][:], scale=-1.0 / WZ_SCALE)
                        # b = (h_pre + bh) * z
                        b_t = bpool.tile([128, 512], F32)
                        nc.vector.scalar_tensor_tensor(
                            b_t[:, :w1 - w0], psh[:, :w1 - w0], bh_t[m][:],
                            z_t[:, :w1 - w0],
                            op0=OP.add, op1=OP.mult,
                        )
                        # h_t = a_t * h_{t-1} + b_t along seq (fp16 out)
                        h_t = hv[:, m, w0:w1]
                        init = 0.0 if last_h[m] is None else last_h[m][:, -1:]
                        nc.vector.tensor_tensor_scan(
                            h_t, a_t[:, :w1 - w0], b_t[:, :w1 - w0], init,
                            op0=OP.mult, op1=OP.add,
                        )
                        last_h[m] = h_t
                # One packed store per chunk on the scalar HWDGE ring; the
                # final chunk splits across sync+scalar so the flush
                # overlaps the last scans.
                if c == nchunk - 1:
                    half = NM * chunk // 2
                    nc.sync.dma_start(
                        out=hTp[:, NM * seq_off:NM * seq_off + half],
                        in_=h_t3[:, :half],
                    )
                    nc.scalar.dma_start(
                        out=hTp[:, NM * seq_off + half:NM * (seq_off + chunk)],
                        in_=h_t3[:, half:NM * chunk],
                    )
                else:
                    nc.scalar.dma_start(
                        out=hTp[:, NM * seq_off:NM * (seq_off + chunk)],
                        in_=h_t3[:, :NM * chunk],
                    )
                seq_off += chunk
    _split_sync_waits(nc)
    return nc


_NC_CACHE = None


def _get_program():
    global _NC_CACHE
    if _NC_CACHE is None:
        _NC_CACHE = _build_program()
    return _NC_CACHE


def _pack_x(xb, np_dtype, chunks):
    """xb: (SEQ, D) one batch. Returns [128, 8*SEQ] packed per chunk:
    row p, chunk c: [x[c0:c1, i*128+p] for i in 0..7] concatenated."""
    seq = xb.shape[0]
    out = np.empty((128, NK * seq), dtype=np_dtype)
    off = 0
    for c in chunks:
        blk = xb[off:off + c, :].astype(np_dtype)         # (c, 1024)
        # (c, 8, 128) -> (128, 8, c)
        blk = blk.reshape(c, NK, 128).transpose(2, 1, 0)
        out[:, NK * off:NK * (off + c)] = blk.reshape(128, NK * c)
        off += c
    return np.ascontiguousarray(out)


def _make_in_maps(x, Wz, bz, Wh, bh):
    import ml_dtypes

    f8np = ml_dtypes.float8_e4m3
    xP16 = [_pack_x(x[b], np.float16, CHUNKS) for b in range(BATCH)]
    xP8 = [_pack_x(x[b], f8np, CHUNKS) for b in range(BATCH)]
    wz8p = []
    for c in range(2):
        w = (Wz[:, c * DH:(c + 1) * DH] * WZ_SCALE).astype(f8np)  # (1024, 512)
        # row 256*kt + 128*i + p -> [p, (2*kt+i)*DH + m]
        w = w.reshape(NK2, 2, 128, DH).transpose(2, 0, 1, 3).reshape(128, -1)
        wz8p.append(np.ascontiguousarray(w))
    whh = [np.ascontiguousarray(Wh[:, c * DH:(c + 1) * DH]).astype(np.float16)
           for c in range(2)]
    # bias[p, m] = bz[m*128+p]; columns [0:NM]=bz, [NM:2NM]=-bz, [2NM:3NM]=bh
    biases = []
    for c in range(2):
        bzc = bz[c * DH:(c + 1) * DH].astype(np.float32).reshape(NM, 128).T
        bhc = bh[c * DH:(c + 1) * DH].astype(np.float32).reshape(NM, 128).T
        biases.append(np.ascontiguousarray(np.hstack([bzc, -bzc, bhc])))
    in_maps = []
    for i in range(N_CORES):
        b, c = i // 2, i % 2
        in_maps.append({
            "xP16": xP16[b], "xP8": xP8[b], "wz8p": wz8p[c], "wh": whh[c],
            "bias": biases[c],
        })
    return in_maps


def _unpack_h(hTp, chunks):
    """hTp: [128, 4*SEQ] fp16 packed -> (SEQ, 512) f32."""
    seq = sum(chunks)
    out = np.empty((seq, DH), dtype=np.float32)
    off = 0
    for c in chunks:
        blk = hTp[:, NM * off:NM * (off + c)].astype(np.float32)
        # (128, 4, c) -> (c, 4, 128) -> (c, 512)
        blk = blk.reshape(128, NM, c).transpose(2, 1, 0).reshape(c, DH)
        out[off:off + c] = blk
        off += c
    return out


def _run(x, Wz, bz, Wh, bh, trace=False, trace_cores=None):
    import time

    nc = _get_program()
    in_maps = _make_in_maps(x, Wz, bz, Wh, bh)
    res = None
    for attempt in range(3):
        try:
            res = run_bass_kernel_spmd(
                nc, in_maps, list(range(N_CORES)),
                trace=trace, trace_cores=trace_cores,
            )
            break
        except Exception:
            # Transient NRT device errors have been observed on the first
            # execution after a fresh compile; retry.
            if attempt == 2:
                raise
            time.sleep(10)
    out = np.empty((BATCH, SEQ, D), dtype=np.float32)
    for i in range(N_CORES):
        b, c = i // 2, i % 2
        out[b, :, c * DH:(c + 1) * DH] = _unpack_h(res.results[i]["hTp"], CHUNKS)
    return out, res


def kernel(x, Wz, bz, Wh, bh):
    x = np.asarray(x, dtype=np.float32)
    Wz = np.asarray(Wz, dtype=np.float32)
    Wh = np.asarray(Wh, dtype=np.float32)
    bz = np.asarray(bz, dtype=np.float32)
    bh = np.asarray(bh, dtype=np.float32)
    out, _ = _run(x, Wz, bz, Wh, bh, trace=False)
    return out
